# revision 29
# baseline (speedup 1.0000x reference)
"""Varlen causal GQA attention (B=4, S=1024, HQ=32, HK=8, D=128, fp32)
on 8 Trainium2 NeuronCores.

Sharding: tensor-parallel over the 8 kv heads (GQA groups stay together):
core i gets kv head i and query heads [4i, 4i+4), all 4 sequences. No
collectives; gather = concat along the head axis on host.

Per-core kernel, per (seq b, head-pair hp) over the full 1024-query span:
  for each 128-key tile kt (live query cols [128*kt, 1024), split at 512):
    scores_T[k,q] = K_tile^T.T @ Q^T     (float32r matmul, one per head)
    P_T = exp(scale * scores_T)          (ScalarE; one strided (2,w) exp
                                          covers both heads, PSUM->fp16)
    P_T[:, :, :128] *= causal triangle   (DVE, shared (128,128) mask)
  per head, per 128-query block qi (two PSUM chains share a bank):
    O[q,:128|128] += P_T_slice.T @ [V|1] (fp16 matmul; col 128 = sum exp)
    O = O[:, :128] * 1/O[:, 128]         (DVE reciprocal + broadcast mul)
The kernel is softmax(ScalarE-exp)-roofline-bound: ~0.85ns/col + ~335ns
per exp instruction; PSUM's 8 banks cap exp batching at (2,512) tiles.
Q/K arrive host-pre-transposed to (d, token) layout; V as fp16.
"""

import numpy as np
import ml_dtypes

import concourse.bass as bass
import concourse.tile as tile
import concourse.mybir as mybir
from concourse import bacc
from concourse.bass_utils import run_bass_kernel_spmd

B, S, D = 4, 1024, 128
HQ, HK = 32, 8
G = HQ // HK          # query heads per kv head (= per core)
N_CORES = 8
SCALE = 1.0 / float(np.sqrt(D))
KTW = 128             # key-tile width (matmul stationary free dim)
KT = S // KTW         # key tiles per sequence
NQI = S // 128        # 128-query blocks per sequence
MMW = 512             # max matmul moving free dim

F32 = mybir.dt.float32
F32R = mybir.dt.float32r
FP16 = mybir.dt.float16


def _score_bins():
    """Bin-pack the ragged live score pieces (kt, half, c0, w) into
    512-col PSUM bank rows. Widths: six 512s + 2x384 + 2x256 + 2x128
    -> exactly 9 bins of <=512."""
    pieces = []
    for kt in range(KT):
        c0 = KTW * kt
        if c0 < MMW:
            pieces.append((kt, 0, c0, MMW - c0))
            pieces.append((kt, 1, MMW, MMW))
        else:
            pieces.append((kt, 1, c0, S - c0))
    pieces.sort(key=lambda p: -p[3])  # first-fit decreasing
    bins = []
    for p in pieces:
        for abin in bins:
            if sum(x[3] for x in abin) + p[3] <= MMW:
                abin.append(p)
                break
        else:
            bins.append([p])
    return bins


SCORE_BINS = _score_bins()
# un-packed baseline: one bin per piece (12 exps per seq/head-pair)
SCORE_PIECES_UNPACKED = [[p] for abin in _score_bins() for p in abin]


def build_nc(repeat: int = 1, qk_dtype=F32R, ablate: str = "",
             mask_on_pool: bool = False, psp_bufs: int = 3, po_bufs: int = 2,
             use_divide: bool = False, bin_pack: bool = True):
    """Build the single-core Bass program (SPMD across 8 cores).

    repeat > 1 wraps the body in a hardware loop — used only for timing
    (marginal wall time per iteration approximates HW kernel time).
    ablate: timing-only variants with reduced work (WRONG results):
      "pv" = halve PV chains; "qk" = even key tiles only; "dve" = skip
      mask/normalize.
    """
    nc = bacc.Bacc(None, target_bir_lowering=False, debug=False)

    qT = nc.dram_tensor("qT", [G, B, D, S], qk_dtype, kind="ExternalInput")
    kT = nc.dram_tensor("kT", [B, D, S], qk_dtype, kind="ExternalInput")
    v = nc.dram_tensor("v", [B, S, D], FP16, kind="ExternalInput")
    mk = nc.dram_tensor("mk", [D, KTW], FP16, kind="ExternalInput")
    o = nc.dram_tensor("o", [B * S, G, D], F32, kind="ExternalOutput")
    # (b, g, p, qi, d) view of the output for per-(b,h) stores
    o_r = o[:].rearrange("(b qi p) g d -> b g p qi d", b=B, qi=NQI, p=128)

    with tile.TileContext(nc) as tc:
        with (
            tc.tile_pool(name="cpool", bufs=1) as cpool,
            tc.tile_pool(name="kpool", bufs=2) as kpool,
            tc.tile_pool(name="vpool", bufs=2) as vpool,
            tc.tile_pool(name="qpool", bufs=4) as qpool,
            tc.tile_pool(name="ppool", bufs=28) as ppool,
            tc.tile_pool(name="opool", bufs=4) as opool,
            tc.tile_pool(name="rpool", bufs=8) as rpool,
            tc.tile_pool(name="psp", bufs=psp_bufs, space="PSUM") as psp,
            tc.tile_pool(name="ps_o", bufs=po_bufs, space="PSUM") as ps_o,
        ):
            # shared causal triangle: mask[kk, q] = 1 iff q >= kk
            mask_t = cpool.tile([128, KTW], FP16)
            nc.sync.dma_start(out=mask_t[:], in_=mk[:])

            def emit_scores_pair(q_ts, kt_t):
                """QK^T + exp + triangle mask for all 8 key tiles of TWO
                heads at once. The ragged live pieces of all key tiles are
                bin-packed into full 512-col PSUM bank rows so one strided
                (2-head, <=512-col) exp covers each packed bank pair — 9
                exps per (seq, head-pair) instead of 12.

                Returns p_tiles[kt] = (half0_entry_or_None, half1_entry);
                entry = (fp16 tile (128, 2, 512), lo) with local col =
                global q col - lo."""
                bins = SCORE_BINS if bin_pack else SCORE_PIECES_UNPACKED
                piece_map = {}
                for abin in bins:
                    ps = psp.tile([128, 2, MMW], F32, tag="ps", name="ps")
                    pt = ppool.tile([128, 2, MMW], FP16, tag="pt", name="pt")
                    off = 0
                    for (kt, half, c0p, w) in abin:
                        for hh in range(2):
                            nc.tensor.matmul(
                                ps[:, hh, off:off + w],
                                lhsT=kt_t[:, kt * KTW:(kt + 1) * KTW],
                                rhs=q_ts[hh][:, c0p:c0p + w],
                                start=True, stop=True,
                            )
                        piece_map[(kt, half)] = (pt, c0p - off)
                        off += w
                    nc.scalar.activation(
                        pt[:, :, 0:off], ps[:, :, 0:off],
                        mybir.ActivationFunctionType.Exp, scale=SCALE,
                    )
                    # triangle mask on each kt's causal-boundary 128 cols
                    mask_eng = nc.gpsimd if mask_on_pool else nc.vector
                    boff = 0
                    for (kt, half, c0p, w) in abin:
                        if c0p == KTW * kt and ablate != "dve":
                            for hh in range(2):
                                mask_eng.tensor_mul(
                                    pt[:, hh, boff:boff + KTW],
                                    pt[:, hh, boff:boff + KTW], mask_t[:])
                        boff += w
                return [
                    (piece_map.get((kt, 0)), piece_map[(kt, 1)])
                    for kt in range(KT)
                ]

            def emit_pv(st, hh):
                """Probs @ [V|1] for one head of a pair, then normalize."""
                b, h0, p_tiles, v_t, o_ts = st
                o_t = o_ts[hh]
                for qih in range(NQI // 2):
                    # two 128-query accumulation chains share one PSUM bank
                    po = ps_o.tile([128, 2, KTW + 1], F32, tag="po", name="po")
                    for q2 in range(2):
                        qi = qih * 2 + q2
                        kts = [kt for kt in range(qi + 1)
                               if p_tiles[kt] is not None]
                        if ablate == "pv":
                            kts = kts[:len(kts) // 2 + 1]
                        for kt in kts:
                            pt, lo = p_tiles[kt][0 if qi < MMW // KTW else 1]
                            nc.tensor.matmul(
                                po[:, q2, :],
                                lhsT=pt[:, hh, qi * KTW - lo:
                                        (qi + 1) * KTW - lo],
                                rhs=v_t[:, kt, :],
                                start=(kt == kts[0]),
                                stop=(kt == kts[-1]),
                            )
                    if ablate != "dve":
                        if use_divide:
                            nc.vector.tensor_tensor(
                                o_t[:, qih * 2:qih * 2 + 2, :],
                                po[:, :, 0:KTW],
                                po[:, :, KTW:KTW + 1].broadcast_to(
                                    [128, 2, KTW]),
                                mybir.AluOpType.divide,
                            )
                        else:
                            rec = rpool.tile([128, 2], F32, tag="rec",
                                             name="rec")
                            nc.vector.reciprocal(rec[:], po[:, :, KTW])
                            nc.vector.tensor_mul(
                                o_t[:, qih * 2:qih * 2 + 2, :],
                                po[:, :, 0:KTW],
                                rec[:, :, None].broadcast_to([128, 2, KTW]),
                            )
                    else:
                        nc.vector.tensor_copy(
                            o_t[:, qih * 2, :], po[:, 0, 0:KTW])
                nc.gpsimd.dma_start(out=o_r[b, h0 + hh], in_=o_t[:])

            def body(_iv=None):
                pending = None  # one-pair-deep software pipeline
                for b in range(B):
                    kt_t = kpool.tile([128, S], qk_dtype, tag="kt", name="kt_t")
                    # first key tile separately so the first matmul can
                    # start before the bulk load lands (shortens the ramp)
                    nc.sync.dma_start(out=kt_t[:, 0:KTW], in_=kT[b][:, 0:KTW])
                    nc.sync.dma_start(out=kt_t[:, KTW:S], in_=kT[b][:, KTW:S])
                    v_t = vpool.tile([128, KT, KTW + 1], FP16, tag="vt", name="v_t")
                    nc.sync.dma_start(
                        out=v_t[:, :, 0:KTW],
                        in_=v[b].rearrange("(kt p) d -> p kt d", p=128),
                    )
                    nc.vector.memset(v_t[:, :, KTW:KTW + 1], 1.0)
                    for hp in range(G // 2):
                        h0 = hp * 2
                        q_ts, o_ts = [], []
                        for hh in range(2):
                            q_t = qpool.tile([128, S], qk_dtype, tag="qt",
                                             name="q_t")
                            # gpsimd queue: overlaps with the kt/v loads
                            # on the sync queue at each (b, pair) ramp
                            nc.gpsimd.dma_start(out=q_t[:, 0:MMW],
                                                in_=qT[h0 + hh, b][:, 0:MMW])
                            nc.gpsimd.dma_start(out=q_t[:, MMW:S],
                                                in_=qT[h0 + hh, b][:, MMW:S])
                            q_ts.append(q_t)
                            o_ts.append(opool.tile([128, NQI, KTW], F32,
                                                   tag="ot", name="o_t"))
                        p_tiles = emit_scores_pair(q_ts, kt_t)
                        if pending is not None:
                            emit_pv(pending, 0)
                            emit_pv(pending, 1)
                        pending = (b, h0, p_tiles, v_t, o_ts)
                if pending is not None:
                    emit_pv(pending, 0)
                    emit_pv(pending, 1)

            if repeat == 1:
                body()
            else:
                with tc.For_i(0, repeat, 1) as iv:
                    body(iv)

    nc.compile()
    return nc


def _build_mask() -> np.ndarray:
    """Shared diagonal-block triangle: mask[kk, q] = 1 iff q >= kk."""
    kk = np.arange(128)[:, None]
    qq = np.arange(KTW)[None, :]
    return (qq >= kk).astype(np.float16)


def _core_inputs(q: np.ndarray, k: np.ndarray, v: np.ndarray,
                 qk_np=np.float32):
    """Slice + lay out per-core inputs. Host-side shard/layout step."""
    mask = _build_mask()
    q5 = q.reshape(B, S, HK, G, D)
    k4 = k.reshape(B, S, HK, D)
    v4 = v.reshape(B, S, HK, D)
    in_maps = []
    for c in range(N_CORES):
        qT = np.ascontiguousarray(
            q5[:, :, c, :, :].transpose(2, 0, 3, 1)).astype(qk_np)  # (G,B,D,S)
        kT = np.ascontiguousarray(
            k4[:, :, c, :].transpose(0, 2, 1)).astype(qk_np)        # (B,D,S)
        vb = np.ascontiguousarray(v4[:, :, c, :]).astype(np.float16)
        in_maps.append({"qT": qT, "kT": kT, "v": vb, "mk": mask})
    return in_maps


_NC_CACHE = {}


def kernel(q, k, v, cu_seqlens_q=None, cu_seqlens_k=None,
           max_seqlen_q=None, max_seqlen_k=None) -> np.ndarray:
    q = np.asarray(q, dtype=np.float32)
    k = np.asarray(k, dtype=np.float32)
    v = np.asarray(v, dtype=np.float32)
    assert q.shape == (B * S, HQ, D) and k.shape == (B * S, HK, D)

    if "nc" not in _NC_CACHE:
        _NC_CACHE["nc"] = build_nc(repeat=1)
    nc = _NC_CACHE["nc"]

    in_maps = _core_inputs(q, k, v)
    res = None
    for attempt in range(3):
        try:
            res = run_bass_kernel_spmd(nc, in_maps,
                                       core_ids=list(range(N_CORES)))
            break
        except Exception:
            # a wedged NeuronCore fails once and resets; retry clean
            if attempt == 2:
                raise
            import time as _time
            _time.sleep(2.0)

    out = np.empty((B * S, HQ, D), np.float32)
    for c in range(N_CORES):
        out[:, c * G:(c + 1) * G, :] = res.results[c]["o"]
    return out
